# revision 1
# baseline (speedup 1.0000x reference)
"""MoD-router FFN kernel for 8 TRN2 NeuronCores (self-contained).

Math note: the reference applies softmax over a size-1 axis, which yields
all-ones scores for ANY input; jax.lax.top_k is stable, so the selected
token indices are always [0..NUM_TOKENS) per batch row. The router weights
(Wp, bp) therefore cannot affect the output, and the kernel computes

    out = gelu_tanh(x[:, :2048, :] @ W1 + b1) @ W2 + b2

Sharding: data-parallel over the 4*2048 = 8192 selected token rows ->
1024 rows per core. Each core runs a fused transposed FFN:
  H^T = gelu(W1^T @ X^T + b1)   (per F-block of 512, kept in SBUF)
  out^T += W2_blk^T @ H^T_blk   (accumulated in SBUF fp32)
Matmuls run in float32r (full PE rate at N=512, ~1.5e-4 rel err).
"""

import numpy as np

B, S, D, F = 4, 4096, 2048, 8192
NUM_TOKENS = 2048
NCORES = 8
ROWS = (B * NUM_TOKENS) // NCORES     # 1024 rows per core
P = 128
KT_D = D // P                         # 16 k-subtiles over D
FT = F // P                           # 64 f-tiles
FB = 16                               # F-blocks of 512
FSUB = 4                              # f-subtiles per block
DT = D // P                           # 16 d-tiles
NCH = ROWS // 512                     # 2 row chunks of 512
KS_W2 = 4                             # k-subtiles per F-block in FFN2

_CACHE = {}


def _build():
    import concourse.bass as bass
    import concourse.mybir as mybir
    import concourse.tile as tile
    from concourse import bacc

    f32 = mybir.dt.float32
    f32r = mybir.dt.float32r

    nc = bacc.Bacc()
    xt = nc.declare_dram_parameter("xt", [KT_D, P, ROWS], f32r, isOutput=False)
    w1 = nc.declare_dram_parameter("w1", [FT, P, KT_D, P], f32r, isOutput=False)
    w2 = nc.declare_dram_parameter("w2", [FB, DT, P, KS_W2, P], f32r, isOutput=False)
    b1 = nc.declare_dram_parameter("b1", [P, FT], f32, isOutput=False)
    b2 = nc.declare_dram_parameter("b2", [P, DT], f32, isOutput=False)
    out = nc.declare_dram_parameter("out", [DT, P, ROWS], f32, isOutput=True)

    with tile.TileContext(nc) as tc:
        with (
            tc.tile_pool(name="resident", bufs=1) as res_pool,
            tc.tile_pool(name="w1p", bufs=4) as w1p,
            tc.tile_pool(name="w2p", bufs=4) as w2p,
            tc.tile_pool(name="htp", bufs=8) as htp,
            tc.tile_pool(name="ps1", bufs=4, space="PSUM") as ps1,
            tc.tile_pool(name="ps2", bufs=4, space="PSUM") as ps2,
        ):
            # resident tiles; DMA issue order matters for startup: the first
            # F-block's weights go first, then XT streams in k order so the
            # k-outer warmup block below can compute behind the DMA wave.
            xt_sb = [res_pool.tile([P, ROWS], f32r, name=f"xt{k}") for k in range(KT_D)]
            w1_warm = [w1p.tile([P, KT_D * P], f32r, name=f"w1t_{ft}", tag="w1t")
                       for ft in range(FSUB)]
            b1_sb = res_pool.tile([P, FT], f32, name="b1sb")
            b2_sb = res_pool.tile([P, DT], f32, name="b2sb")
            # Startup DMAs: simple issue order, first-needed first. All
            # queues share HBM BW; gating/trickle schemes measured worse.
            nc.sync.dma_start(out=b1_sb[:], in_=b1[:])
            nc.sync.dma_start(out=b2_sb[:], in_=b2[:])
            for ft in range(2):
                for i in range(4):
                    nc.sync.dma_start(
                        out=w1_warm[ft][:, i * 4 * P:(i + 1) * 4 * P],
                        in_=w1[ft, :, i * 4:(i + 1) * 4, :].rearrange("p k c -> p (k c)"))
            for k in range(KT_D):
                if k < 2:
                    for i in range(2):
                        nc.sync.dma_start(out=xt_sb[k][:, i * 512:(i + 1) * 512],
                                          in_=xt[k, :, i * 512:(i + 1) * 512])
                else:
                    nc.sync.dma_start(out=xt_sb[k][:], in_=xt[k])
            nc.sync.dma_start(out=w1_warm[2][:], in_=w1[2].rearrange("p k c -> p (k c)"))
            nc.sync.dma_start(out=w1_warm[3][:], in_=w1[3].rearrange("p k c -> p (k c)"))

            # out accumulator, initialized to broadcast b2 (scale=0 trick)
            oacc = [res_pool.tile([P, ROWS], f32, name=f"oacc{d}") for d in range(DT)]
            for d in range(DT):
                nc.scalar.activation(
                    oacc[d][:], xt_sb[0][:].bitcast(f32),
                    mybir.ActivationFunctionType.Identity,
                    bias=b2_sb[:, d:d + 1], scale=0.0,
                )

            for fb in range(FB):
                ht = []
                if fb == 0:
                    # warmup block: k-outer over 4 concurrent psum chains
                    # (2 f-subtiles x 2 row chunks per pass) so matmuls start
                    # as soon as xt_sb[k] lands instead of waiting for all XT.
                    for fs in range(FSUB):
                        ht.append(htp.tile([P, ROWS], f32r, name=f"ht_{fs}", tag="ht"))
                    for half in range(2):
                        chains = [(half * 2 + i, n) for i in range(2) for n in range(NCH)]
                        psums = {
                            c: ps1.tile([P, 512], f32, name=f"ps1w_{c[0]}_{c[1]}", tag="ps1")
                            for c in chains
                        }
                        for k in range(KT_D):
                            for fs, n in chains:
                                nc.tensor.matmul(
                                    psums[(fs, n)][:],
                                    w1_warm[fs][:, k * P:(k + 1) * P],
                                    xt_sb[k][:, n * 512:(n + 1) * 512],
                                    start=(k == 0), stop=(k == KT_D - 1),
                                )
                        for fs, n in chains:
                            nc.scalar.activation(
                                ht[fs][:, n * 512:(n + 1) * 512], psums[(fs, n)][:],
                                mybir.ActivationFunctionType.Gelu_apprx_tanh,
                                bias=b1_sb[:, fs:fs + 1],
                            )
                else:
                    for fs in range(FSUB):
                        ft = fb * FSUB + fs
                        w1_sb = w1p.tile([P, KT_D * P], f32r, name=f"w1t_{ft}", tag="w1t")
                        nc.sync.dma_start(out=w1_sb[:], in_=w1[ft].rearrange("p k c -> p (k c)"))
                        ht_t = htp.tile([P, ROWS], f32r, name=f"ht_{ft}", tag="ht")
                        for n in range(NCH):
                            psum = ps1.tile([P, 512], f32, name=f"ps1_{ft}_{n}", tag="ps1")
                            for k in range(KT_D):
                                nc.tensor.matmul(
                                    psum[:],
                                    w1_sb[:, k * P:(k + 1) * P],
                                    xt_sb[k][:, n * 512:(n + 1) * 512],
                                    start=(k == 0), stop=(k == KT_D - 1),
                                )
                            nc.scalar.activation(
                                ht_t[:, n * 512:(n + 1) * 512], psum[:],
                                mybir.ActivationFunctionType.Gelu_apprx_tanh,
                                bias=b1_sb[:, ft:ft + 1],
                            )
                        ht.append(ht_t)

                for d in range(DT):
                    w2_sb = w2p.tile([P, KS_W2 * P], f32r, name=f"w2t_{fb}_{d}", tag="w2t")
                    nc.sync.dma_start(out=w2_sb[:], in_=w2[fb, d].rearrange("p k c -> p (k c)"))
                    for n in range(NCH):
                        psum2 = ps2.tile([P, 512], f32, name=f"ps2_{fb}_{d}_{n}", tag="ps2")
                        for ks in range(KS_W2):
                            nc.tensor.matmul(
                                psum2[:],
                                w2_sb[:, ks * P:(ks + 1) * P],
                                ht[ks][:, n * 512:(n + 1) * 512],
                                start=(ks == 0), stop=(ks == KS_W2 - 1),
                            )
                        nc.vector.tensor_add(
                            oacc[d][:, n * 512:(n + 1) * 512],
                            oacc[d][:, n * 512:(n + 1) * 512],
                            psum2[:],
                        )

            for d in range(DT):
                for i in range(2):
                    nc.sync.dma_start(out=out[d, :, i * 512:(i + 1) * 512],
                                      in_=oacc[d][:, i * 512:(i + 1) * 512])

    nc.compile()
    return nc


def _get_nc():
    if "nc" not in _CACHE:
        _CACHE["nc"] = _build()
    return _CACHE["nc"]


def kernel(x, Wp, bp, W1, b1, W2, b2, **_unused):
    from concourse.bass_utils import run_bass_kernel_spmd

    x = np.asarray(x, dtype=np.float32)
    W1 = np.asarray(W1, dtype=np.float32)
    W2 = np.asarray(W2, dtype=np.float32)
    b1 = np.asarray(b1, dtype=np.float32)
    b2 = np.asarray(b2, dtype=np.float32)

    # host-side shard + layout prep
    xs = x[:, :NUM_TOKENS, :].reshape(B * NUM_TOKENS, D)         # [8192, 2048]
    w1h = np.ascontiguousarray(
        W1.reshape(KT_D, P, FT, P).transpose(2, 1, 0, 3))        # [ft, p, k, c]
    w2h = np.ascontiguousarray(
        W2.reshape(FB, KS_W2, P, DT, P).transpose(0, 3, 2, 1, 4))  # [fb, d, p, ks, c]
    b1h = np.ascontiguousarray(b1.reshape(FT, P).T)              # [p, ft]
    b2h = np.ascontiguousarray(b2.reshape(DT, P).T)              # [p, d]

    in_maps = []
    for c in range(NCORES):
        xc = xs[c * ROWS:(c + 1) * ROWS]                         # [1024, 2048]
        xth = np.ascontiguousarray(xc.T.reshape(KT_D, P, ROWS))  # [k, p, n]
        in_maps.append({"xt": xth, "w1": w1h, "w2": w2h, "b1": b1h, "b2": b2h})

    nc = _get_nc()
    res = run_bass_kernel_spmd(nc, in_maps, list(range(NCORES)))

    out = np.empty((B * NUM_TOKENS, D), dtype=np.float32)
    for c in range(NCORES):
        oc = res.results[c]["out"]                               # [d, p, n]
        out[c * ROWS:(c + 1) * ROWS] = oc.reshape(D, ROWS).T
    return out.reshape(B, NUM_TOKENS, D)



# revision 7
# speedup vs baseline: 1.0849x; 1.0849x over previous
"""MoD-router FFN kernel for 8 TRN2 NeuronCores (self-contained).

Math note: the reference applies softmax over a size-1 axis, which yields
all-ones scores for ANY input; jax.lax.top_k is stable, so the selected
token indices are always [0..NUM_TOKENS) per batch row. The router weights
(Wp, bp) therefore cannot affect the output, and the kernel computes

    out = gelu_tanh(x[:, :2048, :] @ W1 + b1) @ W2 + b2

Sharding: data-parallel over the 4*2048 = 8192 selected token rows ->
1024 rows per core. Each core runs a fused transposed FFN in bf16
(rel err ~3e-3 vs the 2e-2 gate; PE rate is the same 1 cycle/row as
float32r but DMA traffic halves):
  H^T = gelu(W1^T @ X^T + b1)   (per F-block of 512, kept in SBUF bf16)
  out^T += W2_blk^T @ H^T_blk   (accumulated in SBUF fp32 via DVE)

Schedule (from baseline trace analysis):
  - DMA issues are ~650ns each and serialize per queue; spread them over
    the sync/act/vector queues so the first matmul starts ~10us earlier.
  - fb=0 FFN1 runs k-outer across 8 concurrent psum chains so compute
    starts as soon as xt[0] lands and is never DMA-paced.
  - Steady state is software-pipelined: window fb runs FFN1(fb) then
    FFN2(fb-1), so FFN2 never waits on the gelu of its own window.
  - out[d] DMAs are issued inside the last window right after each d's
    final accumulation, spread across queues (the baseline serialized
    them at the very end, adding ~9us of pure tail).
b2 is applied on the host (it is all-zeros in this problem's inputs).
"""

import numpy as np

B, S, D, F = 4, 4096, 2048, 8192
NUM_TOKENS = 2048
NCORES = 8
ROWS = (B * NUM_TOKENS) // NCORES     # 1024 rows per core
P = 128
KT = D // P                           # 16 k-subtiles over D (FFN1 contraction)
FT = F // P                           # 64 f-tiles
FB = 16                               # F-blocks of 512
FSUB = FT // FB                       # 4 f-subtiles per block
DT = D // P                           # 16 d-tiles
NCH = 2                               # row chunks of 512 (PSUM bank limit)
KS2 = 4                               # k-subtiles per F-block in FFN2

_CACHE = {}


def _build():
    import concourse.bass as bass
    import concourse.mybir as mybir
    import concourse.tile as tile
    from concourse import bacc

    f32 = mybir.dt.float32
    bf16 = mybir.dt.bfloat16
    GELU = mybir.ActivationFunctionType.Gelu_apprx_tanh

    nc = bacc.Bacc()
    # xt[k] : [P, ROWS] k-th 128-slice of X^T (bf16)
    xt = nc.declare_dram_parameter("xt", [KT, P, ROWS], bf16, isOutput=False)
    # w1[fb]: [P, KT*FSUB*P] with column order (k, fs, c) so a k-slice is
    # contiguous (fb=0 streams k-sliced for the warmup).
    w1 = nc.declare_dram_parameter("w1", [FB, P, KT * FSUB * P], bf16, isOutput=False)
    # w2[fb]: [P, DT*KS2*P] with column order (d, ks, c)
    w2 = nc.declare_dram_parameter("w2", [FB, P, DT * KS2 * P], bf16, isOutput=False)
    b1 = nc.declare_dram_parameter("b1", [P, FT], f32, isOutput=False)
    out = nc.declare_dram_parameter("out", [DT, P, ROWS], f32, isOutput=True)

    with tile.TileContext(nc) as tc:
        with (
            tc.tile_pool(name="resident", bufs=1) as res,
            tc.tile_pool(name="w1p", bufs=2) as w1p,
            tc.tile_pool(name="w2p", bufs=2) as w2p,
            tc.tile_pool(name="htp", bufs=8) as htp,
            tc.tile_pool(name="ps", bufs=8, space="PSUM") as ps,
        ):
            xt_sb = [res.tile([P, ROWS], bf16, name=f"xt{k}") for k in range(KT)]
            oacc = [res.tile([P, ROWS], f32, name=f"oacc{d}") for d in range(DT)]
            b1_sb = res.tile([P, FT], f32, name="b1sb")
            w1_sb = [None] * FB
            w2_sb = [None] * FB

            # --- startup DMAs, spread across queues -----------------------
            # sync: xt (k-paced, first-needed first); act: w1[0] k-slices
            # interleaved so slice k lands before the warmup's k-th step.
            w1_sb[0] = w1p.tile([P, KT * FSUB * P], bf16, name="w1t0", tag="w1t")
            CSL = FSUB * P  # columns per k-slice
            for k in range(KT):
                nc.scalar.dma_start(out=w1_sb[0][:, k * CSL:(k + 1) * CSL],
                                 in_=w1[0, :, k * CSL:(k + 1) * CSL])
                if k < 2:  # split the first two xt tiles for earliest start
                    for h in range(2):
                        nc.sync.dma_start(out=xt_sb[k][:, h * 512:(h + 1) * 512],
                                          in_=xt[k, :, h * 512:(h + 1) * 512])
                else:
                    nc.sync.dma_start(out=xt_sb[k][:], in_=xt[k])
            nc.gpsimd.dma_start(out=b1_sb[:], in_=b1[:])
            # prefetch w1[1] (act queue) and w2[0] (gpsimd SWDGE queue)
            w1_sb[1] = w1p.tile([P, KT * FSUB * P], bf16, name="w1t1", tag="w1t")
            nc.scalar.dma_start(out=w1_sb[1][:], in_=w1[1])
            w2_sb[0] = w2p.tile([P, DT * KS2 * P], bf16, name="w2t0", tag="w2t")
            nc.gpsimd.dma_start(out=w2_sb[0][:], in_=w2[0])

            ht = {}  # (fb, fs) -> [P, ROWS] bf16 tile

            def ffn1_warmup():
                # k-outer across all 8 (fs, n) chains: compute starts once
                # xt[0] + w1[0]'s k=0 slice land, and consumes xt[k] at
                # ~1.7us/step vs ~0.8us/step DMA supply.
                chains = [(fs, n) for fs in range(FSUB) for n in range(NCH)]
                psums = {c: ps.tile([P, 512], f32, name=f"ps_w{c[0]}_{c[1]}", tag="ps")
                         for c in chains}
                for k in range(KT):
                    for fs, n in chains:
                        nc.tensor.matmul(
                            psums[(fs, n)][:],
                            w1_sb[0][:, (k * FSUB + fs) * P:(k * FSUB + fs + 1) * P],
                            xt_sb[k][:, n * 512:(n + 1) * 512],
                            start=(k == 0), stop=(k == KT - 1),
                        )
                for fs in range(FSUB):
                    h = htp.tile([P, ROWS], bf16, name=f"ht_0_{fs}", tag="ht")
                    ht[(0, fs)] = h
                    for n in range(NCH):
                        nc.scalar.activation(
                            h[:, n * 512:(n + 1) * 512], psums[(fs, n)][:],
                            GELU, bias=b1_sb[:, fs:fs + 1],
                        )

            def ffn1(fb):
                for fs in range(FSUB):
                    h = htp.tile([P, ROWS], bf16, name=f"ht_{fb}_{fs}", tag="ht")
                    ht[(fb, fs)] = h
                    for n in range(NCH):
                        psum = ps.tile([P, 512], f32, name=f"ps1_{fb}_{fs}_{n}", tag="ps")
                        for k in range(KT):
                            nc.tensor.matmul(
                                psum[:],
                                w1_sb[fb][:, (k * FSUB + fs) * P:(k * FSUB + fs + 1) * P],
                                xt_sb[k][:, n * 512:(n + 1) * 512],
                                start=(k == 0), stop=(k == KT - 1),
                            )
                        nc.scalar.activation(
                            h[:, n * 512:(n + 1) * 512], psum[:],
                            GELU, bias=b1_sb[:, fb * FSUB + fs:fb * FSUB + fs + 1],
                        )

            def ffn2(fb):
                last = fb == FB - 1
                for d in range(DT):
                    for n in range(NCH):
                        psum = ps.tile([P, 512], f32, name=f"ps2_{fb}_{d}_{n}", tag="ps")
                        for ks in range(KS2):
                            nc.tensor.matmul(
                                psum[:],
                                w2_sb[fb][:, (d * KS2 + ks) * P:(d * KS2 + ks + 1) * P],
                                ht[(fb, ks)][:, n * 512:(n + 1) * 512],
                                start=(ks == 0), stop=(ks == KS2 - 1),
                            )
                        dst = oacc[d][:, n * 512:(n + 1) * 512]
                        if fb == 0:
                            nc.vector.tensor_scalar_add(dst, psum[:], 0.0)
                        else:
                            nc.vector.tensor_add(dst, dst, psum[:])
                    if last:
                        # stream the finished d-tile out now, rotating queues
                        q = (nc.sync, nc.scalar, nc.gpsimd)[d % 3]
                        q.dma_start(out=out[d], in_=oacc[d][:])

            # --- pipelined schedule --------------------------------------
            # window fb: [prefetch w1(fb+1), w2(fb)] FFN1(fb) ; FFN2(fb-1)
            ffn1_warmup()
            for fb in range(1, FB):
                if fb + 1 < FB:
                    w1_sb[fb + 1] = w1p.tile([P, KT * FSUB * P], bf16,
                                             name=f"w1t{fb+1}", tag="w1t")
                    nc.scalar.dma_start(out=w1_sb[fb + 1][:], in_=w1[fb + 1])
                w2_sb[fb] = w2p.tile([P, DT * KS2 * P], bf16,
                                     name=f"w2t{fb}", tag="w2t")
                nc.sync.dma_start(out=w2_sb[fb][:], in_=w2[fb])
                ffn1(fb)
                ffn2(fb - 1)
            ffn2(FB - 1)

    nc.compile()
    return nc


def _get_nc():
    if "nc" not in _CACHE:
        _CACHE["nc"] = _build()
    return _CACHE["nc"]


def _prep_inputs(x, W1, b1):
    """Host-side shard + layout prep -> per-core in_maps."""
    import ml_dtypes

    bf = ml_dtypes.bfloat16
    xs = np.asarray(x, np.float32)[:, :NUM_TOKENS, :].reshape(B * NUM_TOKENS, D)
    # w1h[fb, p, (k, fs, c)] = W1[k*128+p, (fb*4+fs)*128+c]
    w1h = np.ascontiguousarray(
        np.asarray(W1, np.float32).reshape(KT, P, FB, FSUB, P)
        .transpose(2, 1, 0, 3, 4).reshape(FB, P, KT * FSUB * P)).astype(bf)
    b1h = np.ascontiguousarray(np.asarray(b1, np.float32).reshape(FT, P).T)
    in_maps = []
    for c in range(NCORES):
        xc = xs[c * ROWS:(c + 1) * ROWS]                        # [1024, 2048]
        xth = np.ascontiguousarray(xc.T.reshape(KT, P, ROWS)).astype(bf)
        in_maps.append({"xt": xth, "w1": w1h, "b1": b1h})
    return in_maps


def _prep_w2(W2):
    import ml_dtypes

    # w2h[fb, p, (d, ks, c)] = W2[(fb*4+ks)*128+p, d*128+c]
    return np.ascontiguousarray(
        np.asarray(W2, np.float32).reshape(FB, KS2, P, DT, P)
        .transpose(0, 2, 3, 1, 4).reshape(FB, P, DT * KS2 * P)
    ).astype(ml_dtypes.bfloat16)


def _gather(results, b2):
    out = np.empty((B * NUM_TOKENS, D), dtype=np.float32)
    for c in range(NCORES):
        oc = results[c]["out"]                                  # [d, p, n]
        out[c * ROWS:(c + 1) * ROWS] = np.asarray(oc, np.float32).reshape(D, ROWS).T
    b2 = np.asarray(b2, np.float32)
    if b2.any():
        out += b2
    return out.reshape(B, NUM_TOKENS, D)


def kernel(x, Wp, bp, W1, b1, W2, b2, **_unused):
    from concourse.bass_utils import run_bass_kernel_spmd

    in_maps = _prep_inputs(x, W1, b1)
    w2h = _prep_w2(W2)
    for m in in_maps:
        m["w2"] = w2h
    nc = _get_nc()
    res = run_bass_kernel_spmd(nc, in_maps, list(range(NCORES)))
    return _gather(res.results, b2)
